# revision 38
# baseline (speedup 1.0000x reference)
"""LocalGNN_DB Trainium2 kernel: data-parallel over batch (8 cores, 1 traj each).

Single pass over t: S(t) streamed from HBM exactly once. Dual-layout diffusion:
  - natural diffusion  (states^T stationary, S moving)  -> u1,z2,z1 in [feat, node]
  - transposed diffusion (S stationary, states^T moving) -> u1T,z1T in [node, feat]
All matmuls in float32r (TF32-like, full PE rate at free-dim>=256), fp32 accumulate.
Layouts chosen so every compute access starts at a 32-aligned partition:
  stA cols: [y1T 0:64 | z1T 64:76 | xT 76:88]
  zc rows:  [x 0:12 | pad | z2 32:44 | z1 44:56 | pad | ones 64]  (H1e zero-padded to match)

The metric here is wall-clock of run_bass_kernel_spmd (NTFF unavailable under
axon), which is ~94% host->device wire transfer at ~44 MB/s — so inputs ship
minimally: S as raw int16 (the 2e-2 gate's bit floor), x as 24-bit fixed point
(int16 hi + int8 residual), output as fp16, all weights packed into one array.
Dequant scales fold into the filter tap weights host-side (tap k absorbs
s^k * sx), so the device program stays input-independent; DVE upcasts to f32
right after DMA and all matmul numerics are f32. fp16/bf16 S is NOT viable:
the unnormalized GSO makes pre-tanh activations ~85 rms, and tanh
zero-crossing elements amplify relative S error ~280x (fp16's 2.4e-4 ->
6.8e-2 final; int16's absolute-uniform noise -> 1.28e-2).
"""
import sys
sys.path.insert(0, "/opt/trn_rl_repo")
import numpy as np


def _enable_jax_compilation_cache():
    # The axon PJRT backend supports executable serialization; caching the
    # compiled shard_map body skips the per-call XLA+NEFF recompile that
    # run_bass_via_pjrt otherwise pays (it builds a fresh jit closure each
    # call).
    try:
        import jax
        jax.config.update("jax_compilation_cache_dir", "/tmp/jax_comp_cache")
        jax.config.update("jax_persistent_cache_min_compile_time_secs", 0.0)
        jax.config.update("jax_persistent_cache_min_entry_size_bytes", 0)
    except Exception:
        pass

_CACHE = {}

B, T, N, G = 8, 64, 256, 12
F1, F2, R1, R2 = 64, 32, 32, 2


def _build():
    import concourse.tile as tile
    from concourse import bacc, mybir
    from concourse.tile import TileContext

    f32 = mybir.dt.float32
    f16 = mybir.dt.float16
    i16 = mybir.dt.int16
    i8 = mybir.dt.int8
    import os
    f32r = mybir.dt.float32r if os.environ.get("MM_FP32R") else mybir.dt.float32
    Tanh = mybir.ActivationFunctionType.Tanh
    Mult = mybir.AluOpType.mult
    Add = mybir.AluOpType.add

    nc = bacc.Bacc("TRN2", target_bir_lowering=False, debug=False, num_devices=8)
    # S(0) is never used: z_k(0) taps are the zero pad, so only t=1..63 ship
    S_d = nc.dram_tensor("S", [T - 1, N, N], i16, kind="ExternalInput")
    # x ships as 24-bit fixed point: int16 high + int8 residual (x = (hi + lo/256)*sx,
    # sx folded into H1e host-side); f32-grade accuracy at 3/4 the bytes
    xh_d = nc.dram_tensor("xh", [T, G, N], i16, kind="ExternalInput")
    xl_d = nc.dram_tensor("xl", [T, G, N], i8, kind="ExternalInput")
    # all weights + bias rows + 12x12 eye packed into one input:
    # rows 0:65 H1e | 65:258 H2e | 258:291 A1e | 291:324 A2e | 324:336 E12
    w_d = nc.dram_tensor("W", [336, F1], f32, kind="ExternalInput")
    out_d = nc.dram_tensor("out", [T, R2, N], f16, kind="ExternalOutput")

    with TileContext(nc) as tc:
        with tc.tile_pool(name="consts", bufs=1) as consts, \
             tc.tile_pool(name="hpool", bufs=3) as hpool, \
             tc.tile_pool(name="spool", bufs=4) as spool, \
             tc.tile_pool(name="states", bufs=3) as states, \
             tc.tile_pool(name="pnat", bufs=2, space="PSUM") as pnat, \
             tc.tile_pool(name="ptr", bufs=1, space="PSUM") as ptr, \
             tc.tile_pool(name="psm", bufs=2, space="PSUM") as psm:

            h1e = consts.tile([65, F1], f32r, tag="h1")
            h2a = consts.tile([128, F2], f32r, tag="h2a")
            h2b = consts.tile([65, F2], f32r, tag="h2b")
            a1e = consts.tile([F2 + 1, R1], f32r, tag="a1")
            a2e = consts.tile([R1 + 1, R2], f32r, tag="a2")
            nc.sync.dma_start(out=h1e, in_=w_d[0:65, :].bitcast(f32r))
            nc.sync.dma_start(out=h2a, in_=w_d[65:193, 0:F2].bitcast(f32r))
            nc.sync.dma_start(out=h2b, in_=w_d[193:258, 0:F2].bitcast(f32r))
            nc.sync.dma_start(out=a1e, in_=w_d[258:291, 0:R1].bitcast(f32r))
            nc.sync.dma_start(out=a2e, in_=w_d[291:324, 0:R2].bitcast(f32r))
            # 12x12 identity for the PE transpose that derives xT from x
            # (saves shipping a separate transposed copy of x)
            e12 = consts.tile([G, G], f32, tag="e12")
            nc.sync.dma_start(out=e12, in_=w_d[324:336, 0:G])

            stA_prev = [None, None]
            stB_prev = [None, None]

            for t in range(T):
                if t > 0:
                    s0h = hpool.tile([128, N], i16, tag="s0h", name="s0h")
                    s1h = hpool.tile([128, N], i16, tag="s1h", name="s1h")
                    nc.sync.dma_start(out=s0h, in_=S_d[t - 1, 0:128, :])
                    nc.sync.dma_start(out=s1h, in_=S_d[t - 1, 128:256, :])
                    s0 = spool.tile([128, N], f32r, tag="s0", name="s0")
                    s1 = spool.tile([128, N], f32r, tag="s1", name="s1")
                    nc.vector.tensor_copy(out=s0[:, :].bitcast(f32), in_=s0h[:, :])
                    nc.vector.tensor_copy(out=s1[:, :].bitcast(f32), in_=s1h[:, :])
                    s_c = [s0, s1]

                stA = [states.tile([128, 88], f32r, tag=f"stA{c}", name=f"stA{c}")
                       for c in (0, 1)]
                stB = [states.tile([128, F1], f32r, tag=f"stB{c}", name=f"stB{c}")
                       for c in (0, 1)]
                zc = states.tile([65, N], f32r, tag="zc", name="zc")
                uca = states.tile([128, N], f32r, tag="uca", name="uca")
                ucb = states.tile([F1 + 1, N], f32r, tag="ucb", name="ucb")
                y2e = states.tile([F2 + 1, N], f32r, tag="y2e", name="y2e")
                ve = states.tile([F2 + 1, N], f32r, tag="ve", name="ve")

                nc.vector.memset(zc[0:32, :].bitcast(f32), 0.0)
                xhh = hpool.tile([G, N], i16, tag="xhh", name="xhh")
                xll = hpool.tile([G, N], i8, tag="xll", name="xll")
                nc.sync.dma_start(out=xhh, in_=xh_d[t, :, :])
                nc.sync.dma_start(out=xll, in_=xl_d[t, :, :])
                xhf = hpool.tile([G, N], f32, tag="xhf", name="xhf")
                xlf = hpool.tile([G, N], f32, tag="xlf", name="xlf")
                nc.vector.tensor_copy(out=xhf[:, :], in_=xhh[:, :])
                nc.vector.tensor_copy(out=xlf[:, :], in_=xll[:, :])
                nc.vector.scalar_tensor_tensor(
                    out=zc[0:G, :].bitcast(f32), in0=xlf[:, :], scalar=1.0 / 256.0,
                    in1=xhf[:, :], op0=Mult, op1=Add)
                # derive xT on device: stA[c][:,76:88] = (x[t] chunk).T via PE
                for c in (0, 1):
                    pxt = psm.tile([128, G], f32, tag="sm", name="pxt")
                    nc.tensor.matmul(out=pxt[:, :],
                                     lhsT=zc[0:G, c * 128:(c + 1) * 128].bitcast(f32),
                                     rhs=e12[:, :], start=True, stop=True)
                    nc.vector.tensor_copy(out=stA[c][:, 76:88].bitcast(f32),
                                          in_=pxt[:, :])
                nc.vector.memset(zc[64:65, :].bitcast(f32), 1.0)
                nc.vector.memset(ucb[64:65, :].bitcast(f32), 1.0)
                nc.vector.memset(y2e[32:33, :].bitcast(f32), 1.0)
                nc.vector.memset(ve[32:33, :].bitcast(f32), 1.0)

                if t == 0:
                    nc.vector.memset(zc[32:64, :].bitcast(f32), 0.0)
                    nc.vector.memset(uca[64:128, :].bitcast(f32), 0.0)
                    nc.vector.memset(ucb[0:64, :].bitcast(f32), 0.0)
                    for c in (0, 1):
                        nc.vector.memset(stA[c][:, 64:76].bitcast(f32), 0.0)
                        nc.vector.memset(stB[c][:, :].bitcast(f32), 0.0)
                else:
                    # natural diffusion -> pA rows: [u1 0:64 | z2 64:76 | z1 76:88]
                    pA = pnat.tile([88, N], f32, tag="natA", name="pA")
                    pB = pnat.tile([F1, N], f32, tag="natB", name="pB")
                    for c in (0, 1):
                        nc.tensor.matmul(out=pA[:, :], lhsT=stA_prev[c][:, :].bitcast(f32),
                                         rhs=s_c[c][:, :].bitcast(f32), start=(c == 0), stop=(c == 1))
                        nc.tensor.matmul(out=pB[:, :], lhsT=stB_prev[c][:, :].bitcast(f32),
                                         rhs=s_c[c][:, :].bitcast(f32), start=(c == 0), stop=(c == 1))
                    # transposed diffusion -> pT cols: [u1T 0:64 | z2T 64:76 | z1T 76:88]
                    pT = [ptr.tile([128, 88], f32, tag=f"pT{n}", name=f"pT{n}")
                          for n in (0, 1)]
                    for n in (0, 1):
                        for c in (0, 1):
                            nc.tensor.matmul(out=pT[n][:, :],
                                             lhsT=s_c[c][:, n * 128:(n + 1) * 128].bitcast(f32),
                                             rhs=stA_prev[c][:, :].bitcast(f32),
                                             start=(c == 0), stop=(c == 1))
                    nc.vector.memset(zc[32:64, :].bitcast(f32), 0.0)
                    nc.vector.tensor_copy(out=zc[32:56, :], in_=pA[64:88, :])
                    nc.vector.tensor_copy(out=uca[64:128, :], in_=pA[0:64, :])
                    nc.vector.tensor_copy(out=ucb[0:64, :], in_=pB[:, :])
                    for n in (0, 1):
                        nc.vector.tensor_copy(out=stA[n][:, 64:76].bitcast(f32), in_=pT[n][:, 76:88])
                        nc.vector.tensor_copy(out=stB[n][:, :].bitcast(f32), in_=pT[n][:, 0:64])

                # layer-1 taps (natural + transposed)
                p1 = psm.tile([F1, N], f32, tag="sm", name="p1")
                nc.tensor.matmul(out=p1[:, :], lhsT=h1e[:, :], rhs=zc[:, :],
                                 start=True, stop=True)
                nc.scalar.activation(out=uca[0:F1, :], in_=p1[:, :], func=Tanh)
                for n in (0, 1):
                    p1t = psm.tile([128, F1], f32, tag="sm", name="p1t")
                    nc.tensor.matmul(out=p1t[:, :], lhsT=zc[:, n * 128:(n + 1) * 128].bitcast(f32),
                                     rhs=h1e[:, :].bitcast(f32), start=True, stop=True)
                    nc.scalar.activation(out=stA[n][:, 0:F1].bitcast(f32), in_=p1t[:, :], func=Tanh)

                # layer-2 taps (natural only)
                p2 = psm.tile([F2, N], f32, tag="sm", name="p2")
                nc.tensor.matmul(out=p2[:, :], lhsT=h2a[:, :], rhs=uca[:, :],
                                 start=True, stop=False)
                nc.tensor.matmul(out=p2[:, :], lhsT=h2b[:, :], rhs=ucb[:, :],
                                 start=False, stop=True)
                nc.scalar.activation(out=y2e[0:F2, :], in_=p2[:, :], func=Tanh)

                # readout
                p3 = psm.tile([R1, N], f32, tag="sm", name="p3")
                nc.tensor.matmul(out=p3[:, :], lhsT=a1e[:, :], rhs=y2e[:, :],
                                 start=True, stop=True)
                nc.scalar.activation(out=ve[0:R1, :], in_=p3[:, :], func=Tanh)
                po = psm.tile([R2, N], f32, tag="sm", name="po")
                nc.tensor.matmul(out=po[:, :], lhsT=a2e[:, :], rhs=ve[:, :],
                                 start=True, stop=True)
                osb = states.tile([R2, N], f16, tag="osb", name="osb")
                nc.scalar.copy(out=osb[:, :], in_=po[:, :])
                nc.sync.dma_start(out=out_d[t, :, :], in_=osb[:, :])

                stA_prev, stB_prev = stA, stB

    nc.compile()
    return nc


def kernel(x, S, W1, b1, W2, b2, A1, c1, A2, c2):
    _enable_jax_compilation_cache()
    from concourse.bass_utils import run_bass_kernel_spmd

    if "nc" not in _CACHE:
        _CACHE["nc"] = _build()
    nc = _CACHE["nc"]

    x = np.asarray(x, dtype=np.float32)
    S = np.asarray(S, dtype=np.float32)
    W1 = np.asarray(W1, np.float32)
    W2 = np.asarray(W2, np.float32)
    # int16 quantization of S; the device computes with raw integer values,
    # so tap k of each filter absorbs s**k (z_k carries k products with S/s).
    s = max(float(np.abs(S).max()), 1e-30) / 32767.0
    Si = np.round(S / s).clip(-32767, 32767).astype(np.int16)
    # 24-bit fixed-point x: device sees raw (hi + lo/256), every layer-1 tap
    # is linear in x, so sx folds into all H1e tap blocks (not the bias)
    sx = max(float(np.abs(x).max()), 1e-30) / 32767.0
    v = x.astype(np.float64) / sx
    xhi = np.round(v)
    xlo = np.round((v - xhi) * 256.0).clip(-127, 127)
    xhi = xhi.astype(np.int16)
    xlo = xlo.astype(np.int8)
    # H1e rows: 0:12 = k0 (x), 32:44 = k2 (z2), 44:56 = k1 (z1), 64 = b1, rest 0
    H1e = np.zeros((65, F1), np.float32)
    H1e[0:G] = W1[:, 0, 0, :].T * sx
    H1e[32:32 + G] = W1[:, 0, 2, :].T * (s * s * sx)
    H1e[44:44 + G] = W1[:, 0, 1, :].T * (s * sx)
    H1e[64] = np.asarray(b1, np.float32).reshape(F1)
    H2e = np.concatenate(
        [np.transpose(W2[:, 0], (1, 2, 0)).reshape(3 * F1, F2),
         np.asarray(b2, np.float32).reshape(1, F2)], axis=0)
    H2e[F1:2 * F1] *= s
    H2e[2 * F1:3 * F1] *= s * s
    # one packed weight array; see w_d layout in _build
    W = np.zeros((336, F1), np.float32)
    W[0:65] = H1e
    W[65:258, 0:F2] = H2e
    W[258:291, 0:R1] = np.concatenate(
        [np.asarray(A1, np.float32).T, np.asarray(c1, np.float32).reshape(1, R1)])
    W[291:324, 0:R2] = np.concatenate(
        [np.asarray(A2, np.float32).T, np.asarray(c2, np.float32).reshape(1, R2)])
    W[324:336, 0:G] = np.eye(G, dtype=np.float32)

    in_maps = []
    for b in range(B):
        in_maps.append({
            "S": np.ascontiguousarray(Si[b, 1:, 0]),
            "xh": np.ascontiguousarray(xhi[b]),
            "xl": np.ascontiguousarray(xlo[b]),
            "W": W,
        })
    _CACHE["in_maps"] = in_maps
    res = run_bass_kernel_spmd(nc, in_maps, core_ids=list(range(B)))
    return np.stack([res.results[b]["out"].astype(np.float32) for b in range(B)],
                    axis=0)

